# revision 59
# baseline (speedup 1.0000x reference)
"""nn_AttentionOpt on 8 Trainium2 NeuronCores.

Data-parallel over batch N=8 (one element per core) with *mask compaction*:
the key/query mask keeps only ~53% of the 1024 positions (max 538 for the
reference inputs), so the host gathers live positions and the device kernel
runs a dense LC=544-wide problem (5 key blocks of [128,128,128,128,32]).
Masked positions are exactly zero in the reference output; the host scatters
the compacted result back and zero-fills the rest.

Device math per core (C=1024, H=16, Dh=64):
  x8/xr       fp8(e4m3) value+residual of compacted seq     (host-prepped)
  W8/Wr       fp8 value+residual of 16*W (scale keeps the residual out of
              fp8 denormals; host-prepped, pre-transposed)
  Q^T,K^T     relu(16*(Wq x) + 16 bq)  kept *16-scaled* in bf16
              via 3-term DoubleRow fp8 matmuls: W8 x8 + W8 xr + Wr x8
  V           relu(Wv x + bv) * mask   bf16, layout [key, d], computed with
              swapped operands (x stationary, Wv^T moving) + mask/16 eviction
  S^T         = K^T_h'.T Q^T_h' per key block (bf16), exp on ACT with
              scale 1/(8*256) undoing both 16x scales
  P           = [mask_rep | V_h]^T E^T  -> rows 0:64 = masked denominator,
              rows 64:128 = numerator (key mask folded into the aug slot and
              the V rows; no score masking, no exp bias needed)
  y_h         = P[64:128] / P[0:64]    (single DVE divide)
  LayerNorm   two-pass over C via f32r ones-matmuls, rstd broadcast by
              PE rank-1 matmuls; output bf16, host casts to f32 + scatters.
"""
import sys

if "/opt/trn_rl_repo" not in sys.path:
    sys.path.insert(0, "/opt/trn_rl_repo")

from contextlib import ExitStack

import numpy as np
import ml_dtypes

import concourse.bass as bass
import concourse.tile as tile
from concourse import bacc, mybir
from concourse.bass_utils import run_bass_kernel_spmd

f32 = mybir.dt.float32
f32r = mybir.dt.float32r
bf16 = mybir.dt.bfloat16
f8 = mybir.dt.float8e4
AF = mybir.ActivationFunctionType
ALU = mybir.AluOpType
DR = mybir.MatmulPerfMode.DoubleRow

np_f8 = ml_dtypes.float8_e4m3
np_bf16 = ml_dtypes.bfloat16

N_CORES = 8
C = 1024
L = 1024
H = 16
DH = 64
P = 128
NCH = C // P           # 8 channel chunks (and 8 head pairs / d-blocks)
EPS = 1e-5
WS = 16.0              # W (and hence Q/K) pre-scale
EXP_SCALE = 1.0 / (8.0 * WS * WS)   # 1/sqrt(dh) / (16*16)
LC_DEFAULT = 544
USE_DIVIDE = False
AV_EVICT = "act"   # act | none (gpsimd cannot access PSUM)

_BUILT = {}
LAST_RESULTS = None


def _splits(total, step):
    """[(offset, size), ...] covering `total` in chunks of `step`."""
    out = []
    o = 0
    while o < total:
        out.append((o, min(step, total - o)))
        o += step
    return out


def _split3(ap2d, mid, inner):
    """[K, mid*inner] AP -> [K, mid, inner] (same bytes, 3-dim free)."""
    ap = list(ap2d.ap)
    assert len(ap) == 2 and ap[1][0] == 1 and ap[1][1] == mid * inner
    return bass.AP(tensor=ap2d.tensor, offset=ap2d.offset,
                   ap=[ap[0], [inner, mid], [1, inner]])


def _rep_free(src, n, at):
    """Insert a stride-0 broadcast dim of size n at free position `at`."""
    ap = list(src.ap)
    return bass.AP(tensor=src.tensor, offset=src.offset,
                   ap=ap[:at] + [[0, n]] + ap[at:])


def _emit(tc, io, LC):
    nc = tc.nc
    NKB = (LC + P - 1) // P
    KBS = [P] * (NKB - 1) + [LC - P * (NKB - 1)]
    QH = LC // 2
    SG = min(NKB, 5)           # key blocks per scores-psum group
    kb_groups = [list(range(g, min(g + SG, NKB))) for g in range(0, NKB, SG)]

    with ExitStack() as ctx:
        # Clear all of PSUM first: power-on PSUM can hold NaN bit patterns,
        # and exp/junk-lane reads of uninitialized banks would poison the
        # first execution (overlaps the DMA lead-in, so effectively free).
        with tc.tile_pool(name="clr", bufs=1, space="PSUM") as clr:
            clrt = clr.tile([P, 8, 512], f32, tag="clrt")
            nc.vector.memset(clrt[:], 0.0)

        persist = ctx.enter_context(tc.tile_pool(name="persist", bufs=1))

        # ---- small constants ---------------------------------------------
        mrep_sb = persist.tile([P, NKB, DH], bf16, tag="mrep_sb")
        nc.sync.dma_start(out=mrep_sb[:], in_=io["mrep"])
        bq16c = persist.tile([P, NCH], f32, tag="bq16c")
        bk16c = persist.tile([P, NCH], f32, tag="bk16c")
        m16c = persist.tile([P, NKB], f32, tag="m16c")
        lnwc = persist.tile([P, NCH], f32, tag="lnwc")
        lnbc = persist.tile([P, NCH], f32, tag="lnbc")
        bv16r = persist.tile([1, C], bf16, tag="bv16r")
        nc.sync.dma_start(out=bq16c[:], in_=io["bq16c"])
        nc.sync.dma_start(out=bk16c[:], in_=io["bk16c"])
        nc.sync.dma_start(out=m16c[:], in_=io["m16c"])
        nc.sync.dma_start(out=lnwc[:], in_=io["lnwc"])
        nc.sync.dma_start(out=lnbc[:], in_=io["lnbc"])
        nc.sync.dma_start(out=bv16r[:], in_=io["bv16r"])

        ones_f = persist.tile([P, 1], f32, tag="ones_f")
        nc.vector.memset(ones_f[:], 1.0)
        ones_col = persist.tile([P, 1], f32r, tag="ones_col")
        nc.vector.tensor_copy(ones_col[:], ones_f[:])
        ones_rf = persist.tile([1, P], f32, tag="ones_rf")
        nc.vector.memset(ones_rf[:], 1.0)
        ones_row = persist.tile([1, P], f32r, tag="ones_row")
        nc.vector.tensor_copy(ones_row[:], ones_rf[:])
        ones_rb = persist.tile([1, P], bf16, tag="ones_rb")
        nc.vector.tensor_copy(ones_rb[:], ones_rf[:])
        eps_col = persist.tile([1, 1], f32, tag="eps_col")
        nc.vector.memset(eps_col[:], EPS)

        # ---- persistent big tensors --------------------------------------
        # DMA order: seq8 + Q/K dblock-0 weights first (unblocks the first
        # projection), then seqr, V weights dhalf 0, remaining blocks.
        seq8 = persist.tile([P, NCH, LC], f8, tag="seq8")
        seqr = persist.tile([P, NCH, LC], f8, tag="seqr")
        wq8 = persist.tile([P, NCH, NCH, P], f8, tag="wq8")
        wqr = persist.tile([P, NCH, NCH, P], f8, tag="wqr")
        wk8 = persist.tile([P, NCH, NCH, P], f8, tag="wk8")
        wkr = persist.tile([P, NCH, NCH, P], f8, tag="wkr")
        wv8 = persist.tile([P, NCH, C], f8, tag="wv8")
        wvr = persist.tile([P, NCH, C], f8, tag="wvr")

        def dma_w_qk(dc):
            nc.sync.dma_start(out=wq8[:, dc], in_=io["wq8"][:, dc])
            nc.sync.dma_start(out=wqr[:, dc], in_=io["wqr"][:, dc])
            nc.sync.dma_start(out=wk8[:, dc], in_=io["wk8"][:, dc])
            nc.sync.dma_start(out=wkr[:, dc], in_=io["wkr"][:, dc])

        nc.sync.dma_start(out=seq8[:], in_=io["seq8"])
        dma_w_qk(0)
        nc.sync.dma_start(out=seqr[:], in_=io["seqr"])
        nc.sync.dma_start(out=wv8[:, :, 0:512], in_=io["wv8"][:, :, 0:512])
        nc.sync.dma_start(out=wvr[:, :, 0:512], in_=io["wvr"][:, :, 0:512])
        dma_w_qk(1)
        dma_w_qk(2)
        nc.sync.dma_start(out=wv8[:, :, 512:C], in_=io["wv8"][:, :, 512:C])
        nc.sync.dma_start(out=wvr[:, :, 512:C], in_=io["wvr"][:, :, 512:C])
        for dc in range(3, NCH):
            dma_w_qk(dc)

        qT = persist.tile([P, NCH, LC], bf16, tag="qT")
        kT = persist.tile([P, NCH, LC], bf16, tag="kT")
        # v: aug layout [key, kb, h, 128]: cols 0:64 = replicated key mask
        # (denominator rows), cols 64:128 = V_h. The mask slots are filled by
        # one DVE broadcast copy from the staged mrep (a direct DMA would be
        # 10k 128-byte descriptors).
        v = persist.tile([P, NKB, H, P], bf16, tag="v")
        for hh in range(H):
            nc.vector.tensor_copy(v[:, :, hh, 0:DH], mrep_sb[:])
        y = persist.tile([P, NCH, LC], f32r, tag="y")
        t1 = persist.tile([P, NCH, LC], f32, tag="t1")

        # ---- PSUM (attention phase only) ---------------------------------
        # Scores go to double-buffered pair tiles (kb pairs -> 2-bank tiles,
        # last odd kb -> 1-bank tiles) so the next unit's matmuls never wait
        # on the previous unit's exp (WAR decoupling). mm tiles are freed by
        # a one-shot Pool/ACT eviction copy, so bufs=2 suffices.
        attn_ctx = ExitStack()
        scp_ctx = ExitStack()
        mm = attn_ctx.enter_context(
            tc.tile_pool(name="mm", bufs=2, space="PSUM"))
        pjp = attn_ctx.enter_context(
            tc.tile_pool(name="pjp", bufs=1, space="PSUM"))
        eT = attn_ctx.enter_context(tc.tile_pool(name="eT", bufs=14))
        rcpp = attn_ctx.enter_context(tc.tile_pool(name="rcpp", bufs=4))
        eTs = {}
        kb_pairs = [(k, k + 1) for k in range(0, NKB - 1, 2)]
        kb_last = NKB - 1 if NKB % 2 else None
        scA = scp_ctx.enter_context(
            tc.tile_pool(name="scA", bufs=2, space="PSUM"))
        scB = scp_ctx.enter_context(
            tc.tile_pool(name="scB", bufs=1, space="PSUM"))


        qb_dr = _splits(QH, 256)    # DoubleRow moving-dim blocks per q half

        def _proj_group(w8, wr, bcol, tgt, dc, qh):
            ps = pjp.tile([P, QH], f32, tag="pj", name=f"pj{dc}{qh}")
            for qo, qn in qb_dr:
                terms = [(w8, seq8), (wr, seq8), (w8, seqr)]
                n = len(terms) * (NCH // 2)
                i = 0
                for wt, xt in terms:
                    for j in range(NCH // 2):
                        nc.tensor.matmul(
                            ps[:, qo:qo + qn],
                            lhsT=wt[:, dc, 2 * j:2 * j + 2, :],
                            rhs=xt[:, 2 * j:2 * j + 2,
                                   qh * QH + qo:qh * QH + qo + qn],
                            start=(i == 0), stop=(i == n - 1),
                            perf_mode=DR)
                        i += 1
            # relu(ps + 16*b) on DVE, stays 16-scaled in bf16
            nc.vector.tensor_scalar(
                out=tgt[:, dc, qh * QH:(qh + 1) * QH], in0=ps[:],
                scalar1=bcol[:, dc:dc + 1], scalar2=0.0,
                op0=ALU.add, op1=ALU.max)

        def q_proj(dc, qh):
            # Q^T is query-major: each half is an independent column slice
            _proj_group(wq8, wqr, bq16c, qT, dc, qh)

        def k_proj(dc):
            # K^T columns are KEY positions: every scores unit reads all of
            # them, so both halves must be projected before any sc(2dc, *)
            _proj_group(wk8, wkr, bk16c, kT, dc, 0)
            _proj_group(wk8, wkr, bk16c, kT, dc, 1)

        def vp1(dhalf, kb):
            kbsz = KBS[kb]
            ps = mm.tile([P, 512], f32, tag="mm", name=f"v{dhalf}{kb}")
            for db in range(2):
                do = dhalf * 512 + db * 256
                nc.tensor.matmul(
                    ps[0:kbsz, db * 256:(db + 1) * 256],
                    lhsT=ones_rb[0:1, 0:kbsz],
                    rhs=bv16r[0:1, do:do + 256],
                    start=True, stop=False)
                for xt, wt in ((seq8, wv8), (seq8, wvr), (seqr, wv8)):
                    for j in range(NCH // 2):
                        nc.tensor.matmul(
                            ps[0:kbsz, db * 256:(db + 1) * 256],
                            lhsT=xt[:, 2 * j:2 * j + 2,
                                    kb * P:kb * P + kbsz],
                            rhs=wt[:, 2 * j:2 * j + 2, do:do + 256],
                            start=False,
                            stop=(xt is seqr and j == NCH // 2 - 1),
                            perf_mode=DR)
            # relu + key mask + 1/16 descale on DVE: max(ps * m/16, 0)
            nc.vector.tensor_scalar(
                out=v[0:kbsz, kb, dhalf * 8:(dhalf + 1) * 8, DH:P],
                in0=_split3(ps[0:kbsz, :], 8, DH),
                scalar1=m16c[0:kbsz, kb:kb + 1],
                scalar2=0.0, op0=ALU.mult, op1=ALU.max)

        def sc1(h, qh):
            hp, hoff = h // 2, (h % 2) * DH
            eTs[h, qh] = eT.tile([P, NKB, QH], bf16, tag="eT",
                                 name=f"eT{h}_{qh}")
            for ka, kb_ in kb_pairs:
                pA = scA.tile([P, 2, 512], f32, tag="scA",
                              name=f"sA{h}{qh}{ka}")
                for i, kk in enumerate((ka, kb_)):
                    nc.tensor.matmul(
                        pA[0:KBS[kk], i, 0:QH],
                        lhsT=kT[hoff:hoff + DH, hp, kk * P:kk * P + KBS[kk]],
                        rhs=qT[hoff:hoff + DH, hp, qh * QH:(qh + 1) * QH],
                        start=True, stop=True)
                nc.scalar.activation(
                    eTs[h, qh][:, ka:ka + 2, :], pA[:, :, 0:QH],
                    AF.Exp, scale=EXP_SCALE)
            if kb_last is not None:
                kk = kb_last
                pB = scB.tile([P, 512], f32, tag="scB", name=f"sB{h}{qh}")
                nc.tensor.matmul(
                    pB[0:KBS[kk], 0:QH],
                    lhsT=kT[hoff:hoff + DH, hp, kk * P:kk * P + KBS[kk]],
                    rhs=qT[hoff:hoff + DH, hp, qh * QH:(qh + 1) * QH],
                    start=True, stop=True)
                nc.scalar.activation(
                    eTs[h, qh][0:KBS[kk], kk, :], pB[0:KBS[kk], 0:QH],
                    AF.Exp, scale=EXP_SCALE)

        def av1(h, qh, drain=False):
            hp, hoff = h // 2, (h % 2) * DH
            ps = mm.tile([P, QH], f32, tag="mm", name=f"av{h}{qh}")
            for kb in range(NKB):
                kbsz = KBS[kb]
                nc.tensor.matmul(
                    ps[:],
                    lhsT=v[0:kbsz, kb, h, :],
                    rhs=eTs[h, qh][0:kbsz, kb, :],
                    start=(kb == 0), stop=(kb == NKB - 1))
            rcp = rcpp.tile([DH, QH], f32, tag="rcp", name=f"rcp{h}{qh}")
            nc.vector.reciprocal_approx_fast(out=rcp[:], in_=ps[0:DH, :])
            nc.vector.tensor_mul(
                y[hoff:hoff + DH, hp, qh * QH:(qh + 1) * QH],
                ps[DH:P, :], rcp[:])

        # ---- schedule -----------------------------------------------------
        # qh-major: all (h, qh=0) score/AV units first, then qh=1. Greedy
        # weave keeps one PE filler between consecutive score units (hiding
        # the scores-PSUM WAR against the previous exp) and lets AVs chase
        # with a small lag. The qh0 LayerNorm chain then overlaps the qh1
        # attention drain, and the two half-chains overlap each other.
        k_done = set()
        q_done = set()
        vp_pend = [(0, kb) for kb in range(NKB)] + [(1, kb) for kb in range(NKB)]
        av_pend = []            # (h, qh, sc_index)
        # 2:1 interleave: qh0 units lead (their LayerNorm half-chain overlaps
        # the qh1 drain) but qh1 exp work trickles in early enough that the
        # ACT engine never gaps between the two halves.
        sc_units = []
        for g in range(H // 2):
            sc_units += [(2 * g, 0), (2 * g + 1, 0), (g, 1)]
        sc_units += [(h, 1) for h in range(H // 2, H)]

        def prereq(unit):
            h, qh = unit
            if h // 2 not in k_done:
                k_done.add(h // 2)
                k_proj(h // 2)
            if (h // 2, qh) not in q_done:
                q_done.add((h // 2, qh))
                q_proj(h // 2, qh)

        def filler(i, next_sc):
            did_proj = False
            if next_sc is not None:
                h, qh = next_sc
                if (h // 2 not in k_done
                        or (h // 2, qh) not in q_done):
                    prereq(next_sc)
                    did_proj = True

            def av_ready():
                if not av_pend or av_pend[0][2] > i - 1:
                    return False
                need_dh = 0 if av_pend[0][0] < 8 else 1
                return all(d != need_dh for d, _ in vp_pend)

            if did_proj:
                return
            if vp_pend and not av_ready():
                vp1(*vp_pend.pop(0))
                return
            keep = 6 if next_sc is not None else 0
            if len(av_pend) > keep and av_ready():
                u = av_pend.pop(0)
                av1(u[0], u[1])
            while len(av_pend) > keep and av_ready():
                u = av_pend.pop(0)
                av1(u[0], u[1])

        prereq(sc_units[0])
        for i, (h, qh) in enumerate(sc_units):
            nxt = sc_units[i + 1] if i + 1 < len(sc_units) else None
            sc1(h, qh)
            av_pend.append((h, qh, i))
            filler(i, nxt)
        # scores PSUM no longer needed; free its 5 banks for the LN stats
        scp_ctx.close()

        # ---- LayerNorm: two overlapping half-chains -----------------------
        tail_ctx = ExitStack()
        tailp = tail_ctx.enter_context(
            tc.tile_pool(name="tailp", bufs=2, space="PSUM"))
        rows = tail_ctx.enter_context(tc.tile_pool(name="rows", bufs=1))
        tsq = tail_ctx.enter_context(tc.tile_pool(name="tsq", bufs=4))
        t2p = tail_ctx.enter_context(tc.tile_pool(name="t2p", bufs=4))
        outp = tail_ctx.enter_context(tc.tile_pool(name="outp", bufs=4))
        out_r = io["out"]
        ps_sy = {}
        ps_var = {}
        u_rep = {}
        r_rep = {}
        u_row = {}
        rstd_row = {}

        def u_mm(cc, qh):
            if qh not in ps_sy:
                ps_sy[qh] = tailp.tile([P, 512], f32, tag=f"tps{qh}",
                                       name=f"ps_sy{qh}")
            nc.tensor.matmul(
                ps_sy[qh][0:1, 0:QH], lhsT=ones_col[:],
                rhs=y[:, cc, qh * QH:(qh + 1) * QH],
                start=(cc == 0), stop=(cc == NCH - 1))

        def u_bcast(qh):
            u_row[qh] = rows.tile([1, QH], f32r, tag=f"u_row{qh}",
                                  name=f"u_row{qh}")
            nc.scalar.mul(u_row[qh][:], ps_sy[qh][0:1, 0:QH], 1.0 / C)
            u_rep[qh] = tailp.tile([P, 512], f32, tag=f"tps{qh}",
                                   name=f"u_rep{qh}")
            nc.tensor.matmul(u_rep[qh][:, 0:QH], lhsT=ones_row[:],
                             rhs=u_row[qh][:], start=True, stop=True)

        def t1sq_var(cc, qh):
            if qh not in ps_var:
                ps_var[qh] = tailp.tile([P, 512], f32, tag=f"tps{qh}",
                                        name=f"ps_var{qh}")
            sl = slice(qh * QH, (qh + 1) * QH)
            nc.vector.tensor_sub(t1[:, cc, sl], y[:, cc, sl].bitcast(f32),
                                 u_rep[qh][:, 0:QH])
            t1s = tsq.tile([P, QH], f32r, tag="t1sq", name=f"t1sq{cc}{qh}")
            nc.scalar.square(t1s[:], t1[:, cc, sl])
            nc.tensor.matmul(
                ps_var[qh][0:1, 0:QH], lhsT=ones_col[:], rhs=t1s[:],
                start=(cc == 0), stop=(cc == NCH - 1))

        def rstd_bcast(qh):
            var_row = rows.tile([1, QH], f32, tag=f"var_row{qh}",
                                name=f"var_row{qh}")
            nc.scalar.mul(var_row[:], ps_var[qh][0:1, 0:QH], 1.0 / C)
            ln_row = rows.tile([1, QH], f32, tag=f"ln_row{qh}",
                               name=f"ln_row{qh}")
            nc.scalar.activation(ln_row[:], var_row[:], AF.Ln,
                                 bias=eps_col[:, 0:1])
            rstd_row[qh] = rows.tile([1, QH], f32r, tag=f"rstd_row{qh}",
                                     name=f"rstd_row{qh}")
            nc.scalar.activation(rstd_row[qh][:], ln_row[:], AF.Exp,
                                 scale=-0.5)
            r_rep[qh] = tailp.tile([P, 512], f32, tag=f"tps{qh}",
                                   name=f"r_rep{qh}")
            nc.tensor.matmul(r_rep[qh][:, 0:QH], lhsT=ones_row[:],
                             rhs=rstd_row[qh][:], start=True, stop=True)

        def t2_out(cc, qh):
            sl = slice(qh * QH, (qh + 1) * QH)
            t2 = t2p.tile([P, QH], f32, tag="t2", name=f"t2_{cc}{qh}")
            nc.vector.scalar_tensor_tensor(
                out=t2[:], in0=t1[:, cc, sl], scalar=lnwc[:, cc:cc + 1],
                in1=r_rep[qh][:, 0:QH], op0=ALU.mult, op1=ALU.mult)
            o_sb = outp.tile([P, QH], bf16, tag="o_sb", name=f"o_{cc}{qh}")
            nc.scalar.activation(o_sb[:], t2[:], AF.Identity,
                                 bias=lnbc[:, cc:cc + 1])
            nc.sync.dma_start(out=out_r[:, cc, sl], in_=o_sb[:])

        # drain qh1 AVs while the qh0 chain starts
        def drain(n):
            for _ in range(min(n, len(av_pend))):
                u = av_pend.pop(0)
                av1(u[0], u[1], drain=True)

        drain(2)
        for cc in range(NCH):
            u_mm(cc, 0)
            drain(1)
        u_bcast(0)
        for cc in range(NCH):
            t1sq_var(cc, 0)
            drain(1)
        drain(len(av_pend))
        for cc in range(NCH):
            u_mm(cc, 1)
        u_bcast(1)
        rstd_bcast(0)
        for cc in range(NCH):
            t1sq_var(cc, 1)
            if cc < 4:
                t2_out(2 * cc, 0)
                t2_out(2 * cc + 1, 0)
        rstd_bcast(1)
        for cc in range(NCH):
            t2_out(cc, 1)
        tail_ctx.close()

        attn_ctx.close()


def _pin_act_table(nc):
    """Constrain the activation-table chooser to the single table containing
    every function this kernel uses so no LoadActFuncSet thrash occurs."""
    from concourse.hw_specs import get_activation_tables
    keep = "natural_log_exp_and_others"
    try:
        tabs = get_activation_tables(nc.m.arch)
    except Exception:
        return
    if keep not in tabs:
        return
    shared = set(tabs[keep])
    for name, funcs in tabs.items():
        if name != keep:
            funcs -= shared


def build(LC=LC_DEFAULT):
    if LC in _BUILT:
        return _BUILT[LC]
    NKB = (LC + P - 1) // P
    nc = bacc.Bacc("TRN2", target_bir_lowering=False, debug=False,
                   num_devices=N_CORES)
    _pin_act_table(nc)
    io = {
        "seq8": nc.dram_tensor("seq8", [P, NCH, LC], f8,
                               kind="ExternalInput").ap(),
        "seqr": nc.dram_tensor("seqr", [P, NCH, LC], f8,
                               kind="ExternalInput").ap(),
        "wq8": nc.dram_tensor("wq8", [P, NCH, NCH, P], f8,
                              kind="ExternalInput").ap(),
        "wqr": nc.dram_tensor("wqr", [P, NCH, NCH, P], f8,
                              kind="ExternalInput").ap(),
        "wk8": nc.dram_tensor("wk8", [P, NCH, NCH, P], f8,
                              kind="ExternalInput").ap(),
        "wkr": nc.dram_tensor("wkr", [P, NCH, NCH, P], f8,
                              kind="ExternalInput").ap(),
        "wv8": nc.dram_tensor("wv8", [P, NCH, C], f8,
                              kind="ExternalInput").ap(),
        "wvr": nc.dram_tensor("wvr", [P, NCH, C], f8,
                              kind="ExternalInput").ap(),
        "bq16c": nc.dram_tensor("bq16c", [P, NCH], f32,
                                kind="ExternalInput").ap(),
        "bk16c": nc.dram_tensor("bk16c", [P, NCH], f32,
                                kind="ExternalInput").ap(),
        "bv16r": nc.dram_tensor("bv16r", [1, C], bf16,
                                kind="ExternalInput").ap(),
        "m16c": nc.dram_tensor("m16c", [P, NKB], f32,
                               kind="ExternalInput").ap(),
        "mrep": nc.dram_tensor("mrep", [P, NKB, DH], bf16,
                               kind="ExternalInput").ap(),
        "lnwc": nc.dram_tensor("lnwc", [P, NCH], f32,
                               kind="ExternalInput").ap(),
        "lnbc": nc.dram_tensor("lnbc", [P, NCH], f32,
                               kind="ExternalInput").ap(),
        "out": nc.dram_tensor("out", [P, NCH, LC], bf16,
                              kind="ExternalOutput").ap(),
    }
    with tile.TileContext(nc) as tc:
        _emit(tc, io, LC)
    nc.compile()
    _BUILT[LC] = nc
    return nc


def _quant_res(a):
    a8 = np.ascontiguousarray(a.astype(np_f8))
    r = np.ascontiguousarray((a - a8.astype(np.float32)).astype(np_f8))
    return a8, r


def _w_qk_layout(w):
    # W^T[c, d] -> [p, dc, cc, j] with c = cc*128+p, d = dc*128+j
    ws = (w * WS).astype(np.float32)
    w8, wr = _quant_res(ws.T)
    def lay(a):
        return np.ascontiguousarray(
            a.reshape(NCH, P, NCH, P).transpose(1, 2, 0, 3))
    return lay(w8), lay(wr)


def _w_v_layout(w):
    ws = (w * WS).astype(np.float32)
    w8, wr = _quant_res(ws.T)
    def lay(a):
        return np.ascontiguousarray(a.reshape(NCH, P, C).transpose(1, 0, 2))
    return lay(w8), lay(wr)


def _col(a):
    return np.ascontiguousarray(a.reshape(NCH, P).T.astype(np.float32))


def pick_lc(mask):
    cnt_max = int(mask.reshape(N_CORES, L).sum(axis=1).max())
    lc = max(LC_DEFAULT, ((cnt_max + 31) // 32) * 32)
    assert lc <= 5 * P, f"mask density too high for this kernel (max {cnt_max})"
    return lc


def make_in_maps(seq, mask, wq, bq, wk, bk, wv, bv, ln_w, ln_b, LC=None):
    seq = np.asarray(seq, dtype=np.float32)
    mask = np.asarray(mask).reshape(N_CORES, L)
    if LC is None:
        LC = pick_lc(mask)
    NKB = (LC + P - 1) // P
    wq8, wqr = _w_qk_layout(np.asarray(wq, np.float32))
    wk8, wkr = _w_qk_layout(np.asarray(wk, np.float32))
    wv8, wvr = _w_v_layout(np.asarray(wv, np.float32))
    shared = {
        "wq8": wq8, "wqr": wqr, "wk8": wk8, "wkr": wkr,
        "wv8": wv8, "wvr": wvr,
        "bq16c": _col(np.asarray(bq, np.float32) * WS),
        "bk16c": _col(np.asarray(bk, np.float32) * WS),
        "bv16r": np.ascontiguousarray(
            (np.asarray(bv, np.float32) * WS).astype(np_bf16).reshape(1, C)),
        "lnwc": _col(np.asarray(ln_w, np.float32)),
        "lnbc": _col(np.asarray(ln_b, np.float32)),
    }
    maps, idxs, cnts = [], [], []
    for i in range(N_CORES):
        idx = np.nonzero(mask[i])[0]
        cnt = len(idx)
        xc = np.zeros((C, LC), np.float32)
        xc[:, :cnt] = seq[i][:, idx]
        x8, xr = _quant_res(xc)
        def lay(a):
            return np.ascontiguousarray(
                a.reshape(NCH, P, LC).transpose(1, 0, 2))
        mfull = np.zeros(NKB * P, np.float32)
        mfull[:cnt] = 1.0
        mcol = mfull.reshape(NKB, P).T
        maps.append({
            "seq8": lay(x8), "seqr": lay(xr),
            "m16c": np.ascontiguousarray(mcol / WS),
            "mrep": np.ascontiguousarray(
                np.repeat(mcol[:, :, None], DH, axis=2).astype(np_bf16)),
            **shared,
        })
        idxs.append(idx)
        cnts.append(cnt)
    return maps, idxs, cnts, LC


def kernel(seq, mask, wq, bq, wk, bk, wv, bv, ln_w, ln_b):
    global LAST_RESULTS
    in_maps, idxs, cnts, LC = make_in_maps(
        seq, mask, wq, bq, wk, bk, wv, bv, ln_w, ln_b)
    nc = build(LC)
    res = run_bass_kernel_spmd(nc, in_maps, list(range(N_CORES)))
    LAST_RESULTS = res
    out = np.zeros((N_CORES, C, L), np.float32)
    for i in range(N_CORES):
        oc = np.asarray(res.results[i]["out"]).astype(np.float32)
        oc = oc.transpose(1, 0, 2).reshape(C, LC)
        out[i][:, idxs[i]] = oc[:, :cnts[i]]
    return out


# revision 66
# speedup vs baseline: 1.0080x; 1.0080x over previous
"""nn_AttentionOpt on 8 Trainium2 NeuronCores.

Data-parallel over batch N=8 (one element per core) with *mask compaction*:
the key/query mask keeps only ~53% of the 1024 positions (max 538 for the
reference inputs), so the host gathers live positions and the device kernel
runs a dense LC=544-wide problem (5 key blocks of [128,128,128,128,32]).
Masked positions are exactly zero in the reference output; the host scatters
the compacted result back and zero-fills the rest.

Device math per core (C=1024, H=16, Dh=64):
  x8/xr       fp8(e4m3) value+residual of compacted seq     (host-prepped)
  W8/Wr       fp8 value+residual of 16*W (scale keeps the residual out of
              fp8 denormals; host-prepped, pre-transposed)
  Q^T,K^T     relu(16*(Wq x) + 16 bq)  kept *16-scaled* in bf16
              via 3-term DoubleRow fp8 matmuls: W8 x8 + W8 xr + Wr x8
  V           relu(Wv x + bv) * mask   bf16, layout [key, d], computed with
              swapped operands (x stationary, Wv^T moving) + mask/16 eviction
  S^T         = K^T_h'.T Q^T_h' per key block (bf16), exp on ACT with
              scale 1/(8*256) undoing both 16x scales
  P           = [mask_rep | V_h]^T E^T  -> rows 0:64 = masked denominator,
              rows 64:128 = numerator (key mask folded into the aug slot and
              the V rows; no score masking, no exp bias needed)
  y_h         = P[64:128] / P[0:64]    (single DVE divide)
  LayerNorm   two-pass over C via f32r ones-matmuls, rstd broadcast by
              PE rank-1 matmuls; output bf16, host casts to f32 + scatters.
"""
import sys

if "/opt/trn_rl_repo" not in sys.path:
    sys.path.insert(0, "/opt/trn_rl_repo")

from contextlib import ExitStack

import numpy as np
import ml_dtypes

import concourse.bass as bass
import concourse.tile as tile
from concourse import bacc, mybir
from concourse.bass_utils import run_bass_kernel_spmd

f32 = mybir.dt.float32
f32r = mybir.dt.float32r
bf16 = mybir.dt.bfloat16
f8 = mybir.dt.float8e4
AF = mybir.ActivationFunctionType
ALU = mybir.AluOpType
DR = mybir.MatmulPerfMode.DoubleRow

np_f8 = ml_dtypes.float8_e4m3
np_bf16 = ml_dtypes.bfloat16

N_CORES = 8
C = 1024
L = 1024
H = 16
DH = 64
P = 128
NCH = C // P           # 8 channel chunks (and 8 head pairs / d-blocks)
EPS = 1e-5
WS = 16.0              # W (and hence Q/K) pre-scale
EXP_SCALE = 1.0 / (8.0 * WS * WS)   # 1/sqrt(dh) / (16*16)
LC_DEFAULT = 544
USE_DIVIDE = False
AV_EVICT = "act"   # act | none (gpsimd cannot access PSUM)

_BUILT = {}
LAST_RESULTS = None


def _splits(total, step):
    """[(offset, size), ...] covering `total` in chunks of `step`."""
    out = []
    o = 0
    while o < total:
        out.append((o, min(step, total - o)))
        o += step
    return out


def _split3(ap2d, mid, inner):
    """[K, mid*inner] AP -> [K, mid, inner] (same bytes, 3-dim free)."""
    ap = list(ap2d.ap)
    assert len(ap) == 2 and ap[1][0] == 1 and ap[1][1] == mid * inner
    return bass.AP(tensor=ap2d.tensor, offset=ap2d.offset,
                   ap=[ap[0], [inner, mid], [1, inner]])


def _rep_free(src, n, at):
    """Insert a stride-0 broadcast dim of size n at free position `at`."""
    ap = list(src.ap)
    return bass.AP(tensor=src.tensor, offset=src.offset,
                   ap=ap[:at] + [[0, n]] + ap[at:])


def _emit(tc, io, LC):
    nc = tc.nc
    NKB = (LC + P - 1) // P
    KBS = [P] * (NKB - 1) + [LC - P * (NKB - 1)]
    QH = LC // 2
    SG = min(NKB, 5)           # key blocks per scores-psum group
    kb_groups = [list(range(g, min(g + SG, NKB))) for g in range(0, NKB, SG)]

    with ExitStack() as ctx:
        # Clear all of PSUM first: power-on PSUM can hold NaN bit patterns,
        # and exp/junk-lane reads of uninitialized banks would poison the
        # first execution (overlaps the DMA lead-in, so effectively free).
        with tc.tile_pool(name="clr", bufs=1, space="PSUM") as clr:
            clrt = clr.tile([P, 8, 512], f32, tag="clrt")
            nc.vector.memset(clrt[:], 0.0)

        persist = ctx.enter_context(tc.tile_pool(name="persist", bufs=1))

        # ---- small constants ---------------------------------------------
        mrep_sb = persist.tile([P, NKB, DH], bf16, tag="mrep_sb")
        nc.sync.dma_start(out=mrep_sb[:], in_=io["mrep"])
        bq16c = persist.tile([P, NCH], f32, tag="bq16c")
        bk16c = persist.tile([P, NCH], f32, tag="bk16c")
        m16c = persist.tile([P, NKB], f32, tag="m16c")
        lnwc = persist.tile([P, NCH], f32, tag="lnwc")
        lnbc = persist.tile([P, NCH], f32, tag="lnbc")
        bv16r = persist.tile([1, C], bf16, tag="bv16r")
        nc.sync.dma_start(out=bq16c[:], in_=io["bq16c"])
        nc.sync.dma_start(out=bk16c[:], in_=io["bk16c"])
        nc.sync.dma_start(out=m16c[:], in_=io["m16c"])
        nc.sync.dma_start(out=lnwc[:], in_=io["lnwc"])
        nc.sync.dma_start(out=lnbc[:], in_=io["lnbc"])
        nc.sync.dma_start(out=bv16r[:], in_=io["bv16r"])

        ones_f = persist.tile([P, 1], f32, tag="ones_f")
        nc.vector.memset(ones_f[:], 1.0)
        ones_col = persist.tile([P, 1], f32r, tag="ones_col")
        nc.vector.tensor_copy(ones_col[:], ones_f[:])
        ones_rf = persist.tile([1, P], f32, tag="ones_rf")
        nc.vector.memset(ones_rf[:], 1.0)
        ones_row = persist.tile([1, P], f32r, tag="ones_row")
        nc.vector.tensor_copy(ones_row[:], ones_rf[:])
        ones_rb = persist.tile([1, P], bf16, tag="ones_rb")
        nc.vector.tensor_copy(ones_rb[:], ones_rf[:])
        eps_col = persist.tile([1, 1], f32, tag="eps_col")
        nc.vector.memset(eps_col[:], EPS)

        # ---- persistent big tensors --------------------------------------
        # DMA order: seq8 + Q/K dblock-0 weights first (unblocks the first
        # projection), then seqr, V weights dhalf 0, remaining blocks.
        seq8 = persist.tile([P, NCH, LC], f8, tag="seq8")
        seqr = persist.tile([P, NCH, LC], f8, tag="seqr")
        wq8 = persist.tile([P, NCH, NCH, P], f8, tag="wq8")
        wqr = persist.tile([P, NCH, NCH, P], f8, tag="wqr")
        wk8 = persist.tile([P, NCH, NCH, P], f8, tag="wk8")
        wkr = persist.tile([P, NCH, NCH, P], f8, tag="wkr")
        wv8 = persist.tile([P, NCH, C], f8, tag="wv8")
        wvr = persist.tile([P, NCH, C], f8, tag="wvr")

        def dma_w_qk(dc):
            nc.sync.dma_start(out=wq8[:, dc], in_=io["wq8"][:, dc])
            nc.sync.dma_start(out=wqr[:, dc], in_=io["wqr"][:, dc])
            nc.sync.dma_start(out=wk8[:, dc], in_=io["wk8"][:, dc])
            nc.sync.dma_start(out=wkr[:, dc], in_=io["wkr"][:, dc])

        nc.sync.dma_start(out=seq8[:], in_=io["seq8"])
        dma_w_qk(0)
        nc.sync.dma_start(out=seqr[:], in_=io["seqr"])
        nc.sync.dma_start(out=wv8[:, :, 0:512], in_=io["wv8"][:, :, 0:512])
        nc.sync.dma_start(out=wvr[:, :, 0:512], in_=io["wvr"][:, :, 0:512])
        dma_w_qk(1)
        dma_w_qk(2)
        nc.sync.dma_start(out=wv8[:, :, 512:C], in_=io["wv8"][:, :, 512:C])
        nc.sync.dma_start(out=wvr[:, :, 512:C], in_=io["wvr"][:, :, 512:C])
        for dc in range(3, NCH):
            dma_w_qk(dc)

        qT = persist.tile([P, NCH, LC], bf16, tag="qT")
        kT = persist.tile([P, NCH, LC], bf16, tag="kT")
        # v: aug layout [key, kb, h, 128]: cols 0:64 = replicated key mask
        # (denominator rows), cols 64:128 = V_h. The mask slots are filled by
        # one DVE broadcast copy from the staged mrep (a direct DMA would be
        # 10k 128-byte descriptors).
        v = persist.tile([P, NKB, H, P], bf16, tag="v")
        for hh in range(H):
            nc.vector.tensor_copy(v[:, :, hh, 0:DH], mrep_sb[:])
        y = persist.tile([P, NCH, LC], f32r, tag="y")
        t1 = persist.tile([P, NCH, LC], f32, tag="t1")

        # ---- PSUM (attention phase only) ---------------------------------
        # Scores go to double-buffered pair tiles (kb pairs -> 2-bank tiles,
        # last odd kb -> 1-bank tiles) so the next unit's matmuls never wait
        # on the previous unit's exp (WAR decoupling). mm tiles are freed by
        # a one-shot Pool/ACT eviction copy, so bufs=2 suffices.
        attn_ctx = ExitStack()
        scp_ctx = ExitStack()
        mm = attn_ctx.enter_context(
            tc.tile_pool(name="mm", bufs=2, space="PSUM"))
        pjp = attn_ctx.enter_context(
            tc.tile_pool(name="pjp", bufs=1, space="PSUM"))
        eT = attn_ctx.enter_context(tc.tile_pool(name="eT", bufs=14))
        rcpp = attn_ctx.enter_context(tc.tile_pool(name="rcpp", bufs=4))
        eTs = {}
        kb_pairs = [(k, k + 1) for k in range(0, NKB - 1, 2)]
        kb_last = NKB - 1 if NKB % 2 else None
        scA = scp_ctx.enter_context(
            tc.tile_pool(name="scA", bufs=2, space="PSUM"))
        scB = scp_ctx.enter_context(
            tc.tile_pool(name="scB", bufs=1, space="PSUM"))


        qb_dr = _splits(QH, 256)    # DoubleRow moving-dim blocks per q half

        def _proj_group(w8, wr, bcol, tgt, dc, qh):
            ps = pjp.tile([P, QH], f32, tag="pj", name=f"pj{dc}{qh}")
            for qo, qn in qb_dr:
                terms = [(w8, seq8), (wr, seq8), (w8, seqr)]
                n = len(terms) * (NCH // 2)
                i = 0
                for wt, xt in terms:
                    for j in range(NCH // 2):
                        nc.tensor.matmul(
                            ps[:, qo:qo + qn],
                            lhsT=wt[:, dc, 2 * j:2 * j + 2, :],
                            rhs=xt[:, 2 * j:2 * j + 2,
                                   qh * QH + qo:qh * QH + qo + qn],
                            start=(i == 0), stop=(i == n - 1),
                            perf_mode=DR)
                        i += 1
            # relu(ps + 16*b) on DVE, stays 16-scaled in bf16
            nc.vector.tensor_scalar(
                out=tgt[:, dc, qh * QH:(qh + 1) * QH], in0=ps[:],
                scalar1=bcol[:, dc:dc + 1], scalar2=0.0,
                op0=ALU.add, op1=ALU.max)

        def q_proj(dc, qh):
            # Q^T is query-major: each half is an independent column slice
            _proj_group(wq8, wqr, bq16c, qT, dc, qh)

        def k_proj(dc):
            # K^T columns are KEY positions: every scores unit reads all of
            # them, so both halves must be projected before any sc(2dc, *)
            _proj_group(wk8, wkr, bk16c, kT, dc, 0)
            _proj_group(wk8, wkr, bk16c, kT, dc, 1)

        def vp1(dhalf, kb):
            kbsz = KBS[kb]
            ps = mm.tile([P, 512], f32, tag="mm", name=f"v{dhalf}{kb}")
            for db in range(2):
                do = dhalf * 512 + db * 256
                nc.tensor.matmul(
                    ps[0:kbsz, db * 256:(db + 1) * 256],
                    lhsT=ones_rb[0:1, 0:kbsz],
                    rhs=bv16r[0:1, do:do + 256],
                    start=True, stop=False)
                for xt, wt in ((seq8, wv8), (seq8, wvr), (seqr, wv8)):
                    for j in range(NCH // 2):
                        nc.tensor.matmul(
                            ps[0:kbsz, db * 256:(db + 1) * 256],
                            lhsT=xt[:, 2 * j:2 * j + 2,
                                    kb * P:kb * P + kbsz],
                            rhs=wt[:, 2 * j:2 * j + 2, do:do + 256],
                            start=False,
                            stop=(xt is seqr and j == NCH // 2 - 1),
                            perf_mode=DR)
            # relu + key mask + 1/16 descale on DVE: max(ps * m/16, 0)
            nc.vector.tensor_scalar(
                out=v[0:kbsz, kb, dhalf * 8:(dhalf + 1) * 8, DH:P],
                in0=_split3(ps[0:kbsz, :], 8, DH),
                scalar1=m16c[0:kbsz, kb:kb + 1],
                scalar2=0.0, op0=ALU.mult, op1=ALU.max)

        def sc1(h, qh):
            hp, hoff = h // 2, (h % 2) * DH
            eTs[h, qh] = eT.tile([P, NKB, QH], bf16, tag="eT",
                                 name=f"eT{h}_{qh}")
            for ka, kb_ in kb_pairs:
                pA = scA.tile([P, 2, 512], f32, tag="scA",
                              name=f"sA{h}{qh}{ka}")
                for i, kk in enumerate((ka, kb_)):
                    nc.tensor.matmul(
                        pA[0:KBS[kk], i, 0:QH],
                        lhsT=kT[hoff:hoff + DH, hp, kk * P:kk * P + KBS[kk]],
                        rhs=qT[hoff:hoff + DH, hp, qh * QH:(qh + 1) * QH],
                        start=True, stop=True)
                nc.scalar.activation(
                    eTs[h, qh][:, ka:ka + 2, :], pA[:, :, 0:QH],
                    AF.Exp, scale=EXP_SCALE)
            if kb_last is not None:
                kk = kb_last
                pB = scB.tile([P, 512], f32, tag="scB", name=f"sB{h}{qh}")
                nc.tensor.matmul(
                    pB[0:KBS[kk], 0:QH],
                    lhsT=kT[hoff:hoff + DH, hp, kk * P:kk * P + KBS[kk]],
                    rhs=qT[hoff:hoff + DH, hp, qh * QH:(qh + 1) * QH],
                    start=True, stop=True)
                nc.scalar.activation(
                    eTs[h, qh][0:KBS[kk], kk, :], pB[0:KBS[kk], 0:QH],
                    AF.Exp, scale=EXP_SCALE)

        def av1(h, qh, drain=False):
            hp, hoff = h // 2, (h % 2) * DH
            ps = mm.tile([P, QH], f32, tag="mm", name=f"av{h}{qh}")
            for kb in range(NKB):
                kbsz = KBS[kb]
                nc.tensor.matmul(
                    ps[:],
                    lhsT=v[0:kbsz, kb, h, :],
                    rhs=eTs[h, qh][0:kbsz, kb, :],
                    start=(kb == 0), stop=(kb == NKB - 1))
            rcp = rcpp.tile([DH, QH], f32, tag="rcp", name=f"rcp{h}{qh}")
            nc.vector.reciprocal_approx_fast(out=rcp[:], in_=ps[0:DH, :])
            nc.vector.tensor_mul(
                y[hoff:hoff + DH, hp, qh * QH:(qh + 1) * QH],
                ps[DH:P, :], rcp[:])

        # ---- schedule -----------------------------------------------------
        # qh-major: all (h, qh=0) score/AV units first, then qh=1. Greedy
        # weave keeps one PE filler between consecutive score units (hiding
        # the scores-PSUM WAR against the previous exp) and lets AVs chase
        # with a small lag. The qh0 LayerNorm chain then overlaps the qh1
        # attention drain, and the two half-chains overlap each other.
        k_done = set()
        q_done = set()
        vp_pend = [(0, kb) for kb in range(NKB)] + [(1, kb) for kb in range(NKB)]
        av_pend = []            # (h, qh, sc_index)
        # 2:1 interleave: qh0 units lead (their LayerNorm half-chain overlaps
        # the qh1 drain) but qh1 exp work trickles in early enough that the
        # ACT engine never gaps between the two halves.
        sc_units = []
        for g in range(H // 2):
            sc_units += [(2 * g, 0), (2 * g + 1, 0), (g, 1)]
        sc_units += [(h, 1) for h in range(H // 2, H)]

        def prereq(unit):
            h, qh = unit
            if h // 2 not in k_done:
                k_done.add(h // 2)
                k_proj(h // 2)
            if (h // 2, qh) not in q_done:
                q_done.add((h // 2, qh))
                q_proj(h // 2, qh)

        def filler(i, next_sc):
            did_proj = False
            if next_sc is not None:
                h, qh = next_sc
                if (h // 2 not in k_done
                        or (h // 2, qh) not in q_done):
                    prereq(next_sc)
                    did_proj = True

            def av_ready():
                if not av_pend or av_pend[0][2] > i - 1:
                    return False
                need_dh = 0 if av_pend[0][0] < 8 else 1
                return all(d != need_dh for d, _ in vp_pend)

            if did_proj:
                return
            if vp_pend and not av_ready():
                vp1(*vp_pend.pop(0))
                return
            keep = 6 if next_sc is not None else 0
            if len(av_pend) > keep and av_ready():
                u = av_pend.pop(0)
                av1(u[0], u[1])
            while len(av_pend) > keep and av_ready():
                u = av_pend.pop(0)
                av1(u[0], u[1])

        prereq(sc_units[0])
        for i, (h, qh) in enumerate(sc_units):
            nxt = sc_units[i + 1] if i + 1 < len(sc_units) else None
            sc1(h, qh)
            av_pend.append((h, qh, i))
            filler(i, nxt)
        # scores PSUM no longer needed; free its 5 banks for the LN stats
        scp_ctx.close()

        # ---- LayerNorm: two overlapping half-chains -----------------------
        tail_ctx = ExitStack()
        tailp = tail_ctx.enter_context(
            tc.tile_pool(name="tailp", bufs=2, space="PSUM"))
        rows = tail_ctx.enter_context(tc.tile_pool(name="rows", bufs=1))
        tsq = tail_ctx.enter_context(tc.tile_pool(name="tsq", bufs=4))
        t2p = tail_ctx.enter_context(tc.tile_pool(name="t2p", bufs=4))
        outp = tail_ctx.enter_context(tc.tile_pool(name="outp", bufs=4))
        out_r = io["out"]
        ps_sy = {}
        ps_var = {}
        u_rep = {}
        r_rep = {}
        u_row = {}
        rstd_row = {}

        def u_mm(cc, qh):
            if qh not in ps_sy:
                ps_sy[qh] = tailp.tile([P, 512], f32, tag=f"tps{qh}",
                                       name=f"ps_sy{qh}")
            nc.tensor.matmul(
                ps_sy[qh][0:1, 0:QH], lhsT=ones_col[:],
                rhs=y[:, cc, qh * QH:(qh + 1) * QH],
                start=(cc == 0), stop=(cc == NCH - 1))

        def u_bcast(qh):
            u_row[qh] = rows.tile([1, QH], f32r, tag=f"u_row{qh}",
                                  name=f"u_row{qh}")
            nc.scalar.mul(u_row[qh][:], ps_sy[qh][0:1, 0:QH], 1.0 / C)
            u_rep[qh] = tailp.tile([P, 512], f32, tag=f"tps{qh}",
                                   name=f"u_rep{qh}")
            nc.tensor.matmul(u_rep[qh][:, 0:QH], lhsT=ones_row[:],
                             rhs=u_row[qh][:], start=True, stop=True)

        def t1sq_var(cc, qh):
            if qh not in ps_var:
                ps_var[qh] = tailp.tile([P, 512], f32, tag=f"tps{qh}",
                                        name=f"ps_var{qh}")
            sl = slice(qh * QH, (qh + 1) * QH)
            nc.vector.tensor_sub(t1[:, cc, sl], y[:, cc, sl].bitcast(f32),
                                 u_rep[qh][:, 0:QH])
            t1s = tsq.tile([P, QH], f32r, tag="t1sq", name=f"t1sq{cc}{qh}")
            nc.gpsimd.tensor_tensor(out=t1s[:], in0=t1[:, cc, sl],
                                    in1=t1[:, cc, sl], op=ALU.mult)
            nc.tensor.matmul(
                ps_var[qh][0:1, 0:QH], lhsT=ones_col[:], rhs=t1s[:],
                start=(cc == 0), stop=(cc == NCH - 1))

        def rstd_bcast(qh):
            var_row = rows.tile([1, QH], f32, tag=f"var_row{qh}",
                                name=f"var_row{qh}")
            nc.scalar.mul(var_row[:], ps_var[qh][0:1, 0:QH], 1.0 / C)
            ln_row = rows.tile([1, QH], f32, tag=f"ln_row{qh}",
                               name=f"ln_row{qh}")
            nc.scalar.activation(ln_row[:], var_row[:], AF.Ln,
                                 bias=eps_col[:, 0:1])
            rstd_row[qh] = rows.tile([1, QH], f32r, tag=f"rstd_row{qh}",
                                     name=f"rstd_row{qh}")
            nc.scalar.activation(rstd_row[qh][:], ln_row[:], AF.Exp,
                                 scale=-0.5)
            r_rep[qh] = tailp.tile([P, 512], f32, tag=f"tps{qh}",
                                   name=f"r_rep{qh}")
            nc.tensor.matmul(r_rep[qh][:, 0:QH], lhsT=ones_row[:],
                             rhs=rstd_row[qh][:], start=True, stop=True)

        def t2_out(cc, qh):
            sl = slice(qh * QH, (qh + 1) * QH)
            t2 = t2p.tile([P, QH], f32, tag="t2", name=f"t2_{cc}{qh}")
            nc.vector.scalar_tensor_tensor(
                out=t2[:], in0=t1[:, cc, sl], scalar=lnwc[:, cc:cc + 1],
                in1=r_rep[qh][:, 0:QH], op0=ALU.mult, op1=ALU.mult)
            o_sb = outp.tile([P, QH], bf16, tag="o_sb", name=f"o_{cc}{qh}")
            nc.scalar.activation(o_sb[:], t2[:], AF.Identity,
                                 bias=lnbc[:, cc:cc + 1])
            nc.sync.dma_start(out=out_r[:, cc, sl], in_=o_sb[:])

        # drain qh1 AVs while the qh0 chain starts
        def drain(n):
            for _ in range(min(n, len(av_pend))):
                u = av_pend.pop(0)
                av1(u[0], u[1], drain=True)

        drain(2)
        for cc in range(NCH):
            u_mm(cc, 0)
            drain(1)
        u_bcast(0)
        for cc in range(NCH):
            t1sq_var(cc, 0)
            drain(1)
        drain(len(av_pend))
        for cc in range(NCH):
            u_mm(cc, 1)
        u_bcast(1)
        rstd_bcast(0)
        for cc in range(NCH):
            t1sq_var(cc, 1)
            if cc < 4:
                t2_out(2 * cc, 0)
                t2_out(2 * cc + 1, 0)
        rstd_bcast(1)
        for cc in range(NCH):
            t2_out(cc, 1)
        tail_ctx.close()

        attn_ctx.close()


def _pin_act_table(nc):
    """Constrain the activation-table chooser to the single table containing
    every function this kernel uses so no LoadActFuncSet thrash occurs."""
    from concourse.hw_specs import get_activation_tables
    keep = "natural_log_exp_and_others"
    try:
        tabs = get_activation_tables(nc.m.arch)
    except Exception:
        return
    if keep not in tabs:
        return
    shared = set(tabs[keep])
    for name, funcs in tabs.items():
        if name != keep:
            funcs -= shared


def build(LC=LC_DEFAULT):
    if LC in _BUILT:
        return _BUILT[LC]
    NKB = (LC + P - 1) // P
    nc = bacc.Bacc("TRN2", target_bir_lowering=False, debug=False,
                   num_devices=N_CORES)
    _pin_act_table(nc)
    io = {
        "seq8": nc.dram_tensor("seq8", [P, NCH, LC], f8,
                               kind="ExternalInput").ap(),
        "seqr": nc.dram_tensor("seqr", [P, NCH, LC], f8,
                               kind="ExternalInput").ap(),
        "wq8": nc.dram_tensor("wq8", [P, NCH, NCH, P], f8,
                              kind="ExternalInput").ap(),
        "wqr": nc.dram_tensor("wqr", [P, NCH, NCH, P], f8,
                              kind="ExternalInput").ap(),
        "wk8": nc.dram_tensor("wk8", [P, NCH, NCH, P], f8,
                              kind="ExternalInput").ap(),
        "wkr": nc.dram_tensor("wkr", [P, NCH, NCH, P], f8,
                              kind="ExternalInput").ap(),
        "wv8": nc.dram_tensor("wv8", [P, NCH, C], f8,
                              kind="ExternalInput").ap(),
        "wvr": nc.dram_tensor("wvr", [P, NCH, C], f8,
                              kind="ExternalInput").ap(),
        "bq16c": nc.dram_tensor("bq16c", [P, NCH], f32,
                                kind="ExternalInput").ap(),
        "bk16c": nc.dram_tensor("bk16c", [P, NCH], f32,
                                kind="ExternalInput").ap(),
        "bv16r": nc.dram_tensor("bv16r", [1, C], bf16,
                                kind="ExternalInput").ap(),
        "m16c": nc.dram_tensor("m16c", [P, NKB], f32,
                               kind="ExternalInput").ap(),
        "mrep": nc.dram_tensor("mrep", [P, NKB, DH], bf16,
                               kind="ExternalInput").ap(),
        "lnwc": nc.dram_tensor("lnwc", [P, NCH], f32,
                               kind="ExternalInput").ap(),
        "lnbc": nc.dram_tensor("lnbc", [P, NCH], f32,
                               kind="ExternalInput").ap(),
        "out": nc.dram_tensor("out", [P, NCH, LC], bf16,
                              kind="ExternalOutput").ap(),
    }
    with tile.TileContext(nc) as tc:
        _emit(tc, io, LC)
    nc.compile()
    _BUILT[LC] = nc
    return nc


def _quant_res(a):
    a8 = np.ascontiguousarray(a.astype(np_f8))
    r = np.ascontiguousarray((a - a8.astype(np.float32)).astype(np_f8))
    return a8, r


def _w_qk_layout(w):
    # W^T[c, d] -> [p, dc, cc, j] with c = cc*128+p, d = dc*128+j
    ws = (w * WS).astype(np.float32)
    w8, wr = _quant_res(ws.T)
    def lay(a):
        return np.ascontiguousarray(
            a.reshape(NCH, P, NCH, P).transpose(1, 2, 0, 3))
    return lay(w8), lay(wr)


def _w_v_layout(w):
    ws = (w * WS).astype(np.float32)
    w8, wr = _quant_res(ws.T)
    def lay(a):
        return np.ascontiguousarray(a.reshape(NCH, P, C).transpose(1, 0, 2))
    return lay(w8), lay(wr)


def _col(a):
    return np.ascontiguousarray(a.reshape(NCH, P).T.astype(np.float32))


def pick_lc(mask):
    cnt_max = int(mask.reshape(N_CORES, L).sum(axis=1).max())
    lc = max(LC_DEFAULT, ((cnt_max + 31) // 32) * 32)
    assert lc <= 5 * P, f"mask density too high for this kernel (max {cnt_max})"
    return lc


def make_in_maps(seq, mask, wq, bq, wk, bk, wv, bv, ln_w, ln_b, LC=None):
    seq = np.asarray(seq, dtype=np.float32)
    mask = np.asarray(mask).reshape(N_CORES, L)
    if LC is None:
        LC = pick_lc(mask)
    NKB = (LC + P - 1) // P
    wq8, wqr = _w_qk_layout(np.asarray(wq, np.float32))
    wk8, wkr = _w_qk_layout(np.asarray(wk, np.float32))
    wv8, wvr = _w_v_layout(np.asarray(wv, np.float32))
    shared = {
        "wq8": wq8, "wqr": wqr, "wk8": wk8, "wkr": wkr,
        "wv8": wv8, "wvr": wvr,
        "bq16c": _col(np.asarray(bq, np.float32) * WS),
        "bk16c": _col(np.asarray(bk, np.float32) * WS),
        "bv16r": np.ascontiguousarray(
            (np.asarray(bv, np.float32) * WS).astype(np_bf16).reshape(1, C)),
        "lnwc": _col(np.asarray(ln_w, np.float32)),
        "lnbc": _col(np.asarray(ln_b, np.float32)),
    }
    maps, idxs, cnts = [], [], []
    for i in range(N_CORES):
        idx = np.nonzero(mask[i])[0]
        cnt = len(idx)
        xc = np.zeros((C, LC), np.float32)
        xc[:, :cnt] = seq[i][:, idx]
        x8, xr = _quant_res(xc)
        def lay(a):
            return np.ascontiguousarray(
                a.reshape(NCH, P, LC).transpose(1, 0, 2))
        mfull = np.zeros(NKB * P, np.float32)
        mfull[:cnt] = 1.0
        mcol = mfull.reshape(NKB, P).T
        maps.append({
            "seq8": lay(x8), "seqr": lay(xr),
            "m16c": np.ascontiguousarray(mcol / WS),
            "mrep": np.ascontiguousarray(
                np.repeat(mcol[:, :, None], DH, axis=2).astype(np_bf16)),
            **shared,
        })
        idxs.append(idx)
        cnts.append(cnt)
    return maps, idxs, cnts, LC


def kernel(seq, mask, wq, bq, wk, bk, wv, bv, ln_w, ln_b):
    global LAST_RESULTS
    in_maps, idxs, cnts, LC = make_in_maps(
        seq, mask, wq, bq, wk, bk, wv, bv, ln_w, ln_b)
    nc = build(LC)
    res = run_bass_kernel_spmd(nc, in_maps, list(range(N_CORES)))
    LAST_RESULTS = res
    out = np.zeros((N_CORES, C, L), np.float32)
    for i in range(N_CORES):
        oc = np.asarray(res.results[i]["out"]).astype(np.float32)
        oc = oc.transpose(1, 0, 2).reshape(C, LC)
        out[i][:, idxs[i]] = oc[:, :cnts[i]]
    return out
